# revision 32
# baseline (speedup 1.0000x reference)
"""PointTDA Trainium2 kernel: FPS + kNN + pooling on 8 NeuronCores.

Self-contained: builds a Bass/Tile program, shards batch 2-per-core,
runs via run_bass_kernel_spmd, finishes the tiny BN/cdist tail on host.

Device outputs per core (per sample s, coord c):
  nx    [6*S]            FPS centroid coords, col = 6*t + 2*c + s
  stats [2,3,2,128,S/8]  seg-reduced max/sum of gathered neighbor coords
  m8    [2,128,S/4]      top-32 "NS score" values per query (for sum of
                         selected neighbor distances on host)
where NS[q, n] = 2*(nx_q . x_n) - |x_n|^2  (monotone in -distance per row).
"""
import sys

sys.path.insert(0, "/opt/trn_rl_repo")

import numpy as np

# Problem constants (hardcoded per harness contract)
B, N, S, K = 16, 8192, 1024, 32
NCORES = 8
SPC = B // NCORES          # samples per core
EPS_BN = 1e-5
FACTOR = 1.0

_PROGRAM_CACHE = {}


def build_program(n=N, s_pts=S, k=K, spc=SPC, unroll=8, nschunk=512):
    """Build the Bacc program. Parameterized so a scaled-down version can be
    simulated in CoreSim. Returns the compiled Bacc object."""
    import concourse.bass as bass
    import concourse.bacc as bacc
    import concourse.bass_isa as bass_isa
    import concourse.mybir as mybir
    import concourse.tile as tile

    F32 = mybir.dt.float32
    U16 = mybir.dt.uint16
    ALU = mybir.AluOpType
    ACTF = mybir.ActivationFunctionType
    AXX = mybir.AxisListType.X

    P = 128                      # partitions
    FP = n // P                  # free elems per partition in FPS layout
    nrounds = k // 8
    nqt = s_pts // P             # query tiles per sample
    nch = n // nschunk           # NS chunks per row-tile
    ngrp = P // 16               # 16-partition gpsimd groups

    nc = bacc.Bacc("TRN2", target_bir_lowering=False, debug=False,
                   num_devices=NCORES)

    # ---- DRAM I/O ----
    # iotb[p, f] = 16384 - global point index; used to tie-break the FPS
    # argmax to the lowest index (matches jnp.argmax) when distances tie
    # bit-exactly.
    iotb_d = nc.dram_tensor("iotb", [P, FP], F32, kind="ExternalInput")
    xyz2_d = nc.dram_tensor("xyz2", [P, 6 * FP], F32, kind="ExternalInput")
    xyzt4_d = nc.dram_tensor("xyzt4", [spc, 4, n], F32, kind="ExternalInput")
    nx_d = nc.dram_tensor("nx", [6 * s_pts], F32, kind="ExternalOutput")
    stats_d = nc.dram_tensor("stats", [spc, 3, 2, P // 16, 16 * nqt], F32,
                             kind="ExternalOutput")
    m8_d = nc.dram_tensor("m8", [spc, P, nqt], F32, kind="ExternalOutput")
    idxd_d = nc.dram_tensor("idxd", [P, k * nqt], U16)   # internal scratch

    with tile.TileContext(nc) as tc:
        with tc.tile_pool(name="glob", bufs=1) as gp:
            # kNN moving tensors, loaded up front (independent of FPS).
            # Both samples in one tile at base partitions 0 / 32 (PE matmul
            # requires stationary/moving base partition in {0, 32, 64}).
            xyzt4 = gp.tile([36, n], F32)
            for s in range(spc):
                nc.sync.dma_start(xyzt4[32 * s:32 * s + 4, :], xyzt4_d[s])

            # ---------------- FPS ----------------
            with tc.tile_pool(name="fps", bufs=1) as fp:
                xyz2 = fp.tile([P, 3, spc, FP], F32)
                d2 = fp.tile([P, spc, FP], F32)
                s2 = fp.tile([P, 3, spc, FP], F32)
                t2 = fp.tile([P, spc, FP], F32)
                m2 = fp.tile([P, spc], F32)
                gm = fp.tile([P, spc], F32)
                sel = fp.tile([P, spc, FP], F32)
                m2i = fp.tile([P, spc], F32)
                gsel = fp.tile([P, spc], F32)
                iotb = fp.tile([P, FP], F32)
                msk = fp.tile([P, spc, FP], F32)
                prd = fp.tile([P, 3, spc, FP], F32)
                red = fp.tile([P, 3 * spc], F32)
                cent = fp.tile([P, 3 * spc], F32)
                nxacc = fp.tile([P, 6 * s_pts], F32)

                nc.sync.dma_start(xyz2[:], xyz2_d[:])
                nc.sync.dma_start(iotb[:], iotb_d[:])
                nc.vector.memset(d2[:], 1e10)
                # step-0 centroid = coords of point index 0 (partition 0, f 0)
                nc.gpsimd.partition_broadcast(
                    cent[:],
                    xyz2[0:1, :, :, 0:1].rearrange("o c s f -> o (c s f)"),
                    channels=P)

                def fps_body(iv):
                    base = iv * (3 * spc)
                    # record current centroid (dynamic OUT is HW-safe;
                    # dynamic bias/in operands are NOT)
                    nc.scalar.activation(
                        nxacc[:, bass.ds(base, 3 * spc)], cent[:],
                        ACTF.Copy, scale=1.0)
                    # squares (ACT): s2 = (cent - x)^2, bias from static cols
                    for j in range(3 * spc):
                        nc.scalar.activation(
                            s2[:].rearrange("p c s f -> p (c s) f")[:, j, :],
                            xyz2[:].rearrange("p c s f -> p (c s) f")[:, j, :],
                            ACTF.Square,
                            bias=cent[:, j:j + 1], scale=-1.0)
                    # dist = sx + sy + sz (DVE), min-update (POOL)
                    nc.vector.tensor_tensor(t2[:], s2[:, 0], s2[:, 1], ALU.add)
                    nc.vector.tensor_tensor(t2[:], t2[:], s2[:, 2], ALU.add)
                    nc.vector.tensor_tensor(d2[:], d2[:], t2[:], ALU.min)
                    # argmax: per-partition max, cross-partition allreduce
                    nc.vector.tensor_reduce(m2[:], d2[:], axis=AXX, op=ALU.max)
                    nc.gpsimd.partition_all_reduce(
                        gm[:], m2[:], channels=P,
                        reduce_op=bass_isa.ReduceOp.max)
                    # tie-break to lowest global index: sel = (D>=gmax)*(B-n)
                    for s in range(spc):
                        nc.vector.scalar_tensor_tensor(
                            sel[:, s], d2[:, s], gm[:, s:s + 1], iotb[:],
                            op0=ALU.is_ge, op1=ALU.mult)
                    nc.vector.tensor_reduce(m2i[:], sel[:], axis=AXX,
                                            op=ALU.max)
                    nc.gpsimd.partition_all_reduce(
                        gsel[:], m2i[:], channels=P,
                        reduce_op=bass_isa.ReduceOp.max)
                    # extract winning point's coords via mask + allreduce-add
                    nc.vector.tensor_tensor(
                        msk[:], sel[:],
                        gsel[:].unsqueeze(-1).broadcast_to([P, spc, FP]),
                        ALU.is_ge)
                    nc.vector.tensor_tensor(
                        prd[:], xyz2[:],
                        msk[:].unsqueeze(1).broadcast_to([P, 3, spc, FP]),
                        ALU.mult)
                    nc.vector.tensor_reduce(
                        red[:].rearrange("p (c s) -> p c s", c=3),
                        prd[:], axis=AXX, op=ALU.add)
                    nc.gpsimd.partition_all_reduce(
                        cent[:], red[:], channels=P,
                        reduce_op=bass_isa.ReduceOp.add)

                tc.For_i_unrolled(0, s_pts, 1, fps_body, max_unroll=unroll)
                # dump centroids (row 0; all rows identical)
                nc.sync.dma_start(nx_d[:], nxacc[0:1, 0:6 * s_pts])

            # ---------------- kNN + pooling ----------------
            with tc.tile_pool(name="knn", bufs=1) as kp, \
                 tc.tile_pool(name="knn2", bufs=2) as kp2, \
                 tc.tile_pool(name="psum", bufs=4, space="PSUM") as pp:
                ns = kp.tile([P, n], F32)
                xbc = [kp.tile([P, n], F32, tag=f"xbc{c}", name=f"xbc{c}")
                       for c in range(3)]
                ones1 = kp.tile([1, P], F32)
                nc.vector.memset(ones1[:], 1.0)

                nxt4 = kp.tile([36, s_pts], F32)
                for s in range(spc):
                    nxs = nxt4[32 * s:32 * s + 4, :]
                    nxv = nx_d[:].rearrange("(t c s) -> s c t", t=s_pts, c=3,
                                            s=spc)
                    nc.sync.dma_start(nxs[0:3, :], nxv[s])
                    # row 3 of the stationary = -0.5 (scales |x|^2 row)
                    nc.vector.memset(ns[0:1, 0:s_pts], -0.5)
                    nc.sync.dma_start(nxs[3:4, :], ns[0:1, 0:s_pts])

                    # replicated coord arrays for gathers (stage coord rows
                    # through ns[0:1], which is free until the NS chunks)
                    for c in range(3):
                        nc.sync.dma_start(ns[0:1, :], xyzt4_d[s, c:c + 1])
                        for ch in range(nch):
                            pb = pp.tile([P, nschunk], F32, tag="ps")
                            sl = slice(ch * nschunk, (ch + 1) * nschunk)
                            nc.tensor.matmul(pb[:], ones1[:], ns[0:1, sl],
                                             start=True, stop=True)
                            nc.scalar.activation(xbc[c][:, sl], pb[:],
                                                 ACTF.Copy, scale=1.0)

                    idx = kp.tile([P, k * nqt], U16, tag="idx")
                    iw = kp.tile([P, k * nqt], U16, tag="iw")
                    m8b = kp.tile([P, k * nqt], F32, tag="m8b")
                    m8s = kp.tile([P, nqt], F32, tag="m8s")
                    rstat = [[kp.tile([P, 16 * nqt], F32, tag=f"rs{c}_{st}",
                                      name=f"rs{c}_{st}")
                              for st in range(2)] for c in range(3)]

                    for t in range(nqt):
                        qs = slice(t * P, (t + 1) * P)
                        for ch in range(nch):
                            pb = pp.tile([P, nschunk], F32, tag="ps")
                            sl = slice(ch * nschunk, (ch + 1) * nschunk)
                            nc.tensor.matmul(pb[:], nxs[:, qs],
                                             xyzt4[32 * s:32 * s + 4, sl],
                                             start=True, stop=True)
                            nc.vector.tensor_scalar(ns[:, sl], pb[:], 2.0,
                                                    None, ALU.mult)
                        # top-k rounds (match_replace in place)
                        for r in range(nrounds):
                            mv = m8b[:, k * t + 8 * r: k * t + 8 * r + 8]
                            nc.vector.max(mv, ns[:])
                            nc.vector.max_index(
                                idx[:, k * t + 8 * r: k * t + 8 * r + 8],
                                mv, ns[:])
                            if r < nrounds - 1:
                                nc.vector.match_replace(ns[:], mv, ns[:],
                                                        -3e38)
                        # per-query sum of the top-k scores (for host std)
                        nc.vector.tensor_reduce(
                            m8s[:, t:t + 1], m8b[:, k * t:k * (t + 1)],
                            axis=AXX, op=ALU.add)

                    # wrap indices into gpsimd group-shared layout via DRAM
                    nc.sync.dma_start(idxd_d[:], idx[:])
                    njh = k // 16
                    for t in range(nqt):
                        rsrc = idxd_d[:].rearrange(
                            "(g l) (t jh jl) -> g jl (t jh) l",
                            g=ngrp, l=16, t=nqt, jh=njh, jl=16)
                        rdst = iw[:].rearrange(
                            "p (t l jh) -> p t jh l", t=nqt, l=16, jh=njh)
                        for jh in range(njh):
                            for g in range(ngrp):
                                nc.sync.dma_start(
                                    rdst[16 * g:16 * (g + 1), t, jh, :],
                                    rsrc[g, :, t * njh + jh, :])

                    for t in range(nqt):
                        isl = iw[:, k * t: k * (t + 1)]
                        for c in range(3):
                            g = kp2.tile([P, 16 * k], F32, tag="g")
                            nc.gpsimd.indirect_copy(
                                g[:], xbc[c][:], isl,
                                i_know_ap_gather_is_preferred=True)
                            gv = g[:].rearrange("p (j kk) -> p j kk", j=16)
                            nc.vector.tensor_reduce(
                                rstat[c][0][:, 16 * t:16 * (t + 1)], gv,
                                axis=AXX, op=ALU.max)
                            nc.vector.tensor_reduce(
                                rstat[c][1][:, 16 * t:16 * (t + 1)], gv,
                                axis=AXX, op=ALU.add)

                    for c in range(3):
                        for st in range(2):
                            # one row per 16-partition group (dedup the 16x
                            # replication before download)
                            src = rstat[c][st][:].rearrange(
                                "(g l) f -> g l f", l=16)[:, 0:1, :]
                            nc.sync.dma_start(stats_d[s, c, st], src)
                    nc.sync.dma_start(m8_d[s], m8s[:])

    nc.compile()
    return nc


def _get_program():
    if "full" not in _PROGRAM_CACHE:
        _PROGRAM_CACHE["full"] = build_program()
    return _PROGRAM_CACHE["full"]


def host_prep(xyz):
    """Per-core input layouts. xyz: [B, N, 3] float32."""
    fp = N // 128
    iotb = (16384.0 - (np.arange(128)[:, None] * fp
                       + np.arange(fp)[None, :])).astype(np.float32)
    in_maps = []
    for core in range(NCORES):
        xs = xyz[SPC * core: SPC * (core + 1)]          # [spc, N, 3]
        a = xs.reshape(SPC, 128, N // 128, 3)           # s p f c
        xyz2 = np.transpose(a, (1, 3, 0, 2)).reshape(128, 6 * (N // 128))
        xyzt4 = np.empty((SPC, 4, N), np.float32)
        for s in range(SPC):
            xyzt4[s, 0:3] = xs[s].T
            xyzt4[s, 3] = (xs[s].astype(np.float32) ** 2).sum(-1)
        in_maps.append({"xyz2": np.ascontiguousarray(xyz2),
                        "xyzt4": xyzt4, "iotb": iotb})
    return in_maps


def host_post(results, bn1_gamma, bn1_beta, bn2_gamma, bn2_beta,
              n=N, s_pts=S, k=K):
    """Combine per-core device outputs into the final [B, 36] features."""
    nqt = s_pts // 128
    nxs, maxg, sumg, m8v = [], [], [], []
    for r in results:
        nx = r["nx"].reshape(s_pts, 3, SPC)              # t c s
        stats = r["stats"]                               # [spc,3,2,8,16*nqt]
        m8 = r["m8"]                                     # [spc,128,nqt]
        for s in range(SPC):
            nxs.append(nx[:, :, s])                      # [S, 3]
            # stats group row g, col 16t+j -> query 128t+16g+j
            st = stats[s].reshape(3, 2, 8, nqt, 16)
            st = np.transpose(st, (0, 1, 3, 2, 4)).reshape(3, 2, s_pts)
            maxg.append(st[:, 0].T)                      # [S, 3]
            sumg.append(st[:, 1].T)
            # m8 sums: row p, col t -> query 128t+p
            m8v.append(m8[s].T.reshape(s_pts))

    nx = np.stack(nxs)            # [B, S, 3]
    mxg = np.stack(maxg)          # [B, S, 3]
    smg = np.stack(sumg)          # [B, S, 3]
    m8v = np.stack(m8v)           # [B, S] sum of top-k scores

    # global std of diff (ddof=1) from sums
    sum_d = (smg - k * nx).sum(dtype=np.float64)
    nx2 = (nx.astype(np.float64) ** 2).sum(-1)           # |a|^2 [B,S]
    sum_d2 = (k * nx2 - m8v.astype(np.float64)).sum()
    M = B * s_pts * k * 3
    var = (sum_d2 - sum_d * sum_d / M) / (M - 1)
    std = np.sqrt(max(var, 0.0))

    maxdiff = mxg - nx
    meandiff = smg / k - nx
    p3 = (maxdiff + meandiff) / (std + 1e-5)
    lc = np.concatenate([p3, 2.0 * nx], axis=2)          # [B, S, 6]
    lc = np.transpose(lc, (0, 2, 1)).astype(np.float32)  # [B, 6, S]

    mean = lc.mean(axis=(0, 2), keepdims=True)
    varr = lc.var(axis=(0, 2), keepdims=True)
    lc = (lc - mean) / np.sqrt(varr + EPS_BN) \
        * bn1_gamma[None, :, None] + bn1_beta[None, :, None]
    lc = np.maximum(lc, 0.0)

    # safe cdist over channel rows -> [B, 6, 6]
    x64 = lc.astype(np.float64)
    g = np.einsum("bcs,bds->bcd", x64, x64)
    sq = np.maximum(g.diagonal(axis1=1, axis2=2)[:, :, None]
                    + g.diagonal(axis1=1, axis2=2)[:, None, :] - 2 * g, 0.0)
    # recompute exactly like reference for better fidelity
    d = x64[:, :, None, :] - x64[:, None, :, :]
    sq = (d * d).sum(-1)
    tfcw = np.where(sq > 0, np.sqrt(np.where(sq > 0, sq, 1.0)), 0.0)
    i = np.arange(5)
    tfcw[:, i, i + 1] *= FACTOR

    feat = tfcw.reshape(B, -1)
    mean2 = feat.mean(axis=0, keepdims=True)
    var2 = feat.var(axis=0, keepdims=True)
    feat = (feat - mean2) / np.sqrt(var2 + EPS_BN) \
        * bn2_gamma[None, :] + bn2_beta[None, :]
    feat = np.maximum(feat, 0.0)
    return feat.astype(np.float32)


def _get_runner():
    """Cached jitted shard_map runner over the 8 cores (mirrors
    run_bass_kernel_spmd's axon path, but without per-call retracing)."""
    if "runner" in _PROGRAM_CACHE:
        return _PROGRAM_CACHE["runner"]

    import jax
    import concourse.mybir as mybir
    from concourse import bass2jax
    from jax.sharding import Mesh, PartitionSpec
    from jax.experimental.shard_map import shard_map

    nc = _get_program()
    bass2jax.install_neuronx_cc_hook()
    partition_name = (nc.partition_id_tensor.name
                      if nc.partition_id_tensor else None)
    in_names, out_names, out_avals, zero_shapes = [], [], [], []
    for alloc in nc.m.functions[0].allocations:
        if not isinstance(alloc, mybir.MemoryLocationSet):
            continue
        name = alloc.memorylocations[0].name
        if alloc.kind == "ExternalInput":
            if name != partition_name:
                in_names.append(name)
        elif alloc.kind == "ExternalOutput":
            out_names.append(name)
            shape = tuple(alloc.tensor_shape)
            dtype = mybir.dt.np(alloc.dtype)
            out_avals.append(jax.core.ShapedArray(shape, dtype))
            zero_shapes.append((shape, dtype))
    n_params = len(in_names)
    n_outs = len(out_avals)
    in_names_full = (in_names + out_names
                     + ([partition_name] if partition_name else []))
    donate = tuple(range(n_params, n_params + n_outs))

    def _body(*args):
        operands = list(args)
        if partition_name is not None:
            operands.append(bass2jax.partition_id_tensor())
        outs = bass2jax._bass_exec_p.bind(
            *operands, out_avals=tuple(out_avals),
            in_names=tuple(in_names_full), out_names=tuple(out_names),
            lowering_input_output_aliases=(), sim_require_finite=True,
            sim_require_nnan=True, nc=nc)
        return tuple(outs)

    devices = jax.devices()[:NCORES]
    mesh = Mesh(np.asarray(devices), ("core",))
    sharded = jax.jit(
        shard_map(_body, mesh=mesh,
                  in_specs=(PartitionSpec("core"),) * (n_params + n_outs),
                  out_specs=(PartitionSpec("core"),) * n_outs,
                  check_rep=False),
        donate_argnums=donate, keep_unused=True)

    def run(in_maps):
        concat_in = [np.concatenate([np.asarray(in_maps[c][nm])
                                     for c in range(NCORES)], axis=0)
                     for nm in in_names]
        zeros = [np.zeros((NCORES * sh[0], *sh[1:]), dt)
                 for sh, dt in zero_shapes]
        out = sharded(*concat_in, *zeros)
        return [{name: np.asarray(out[i]).reshape(NCORES,
                                                  *out_avals[i].shape)[c]
                 for i, name in enumerate(out_names)}
                for c in range(NCORES)]

    _PROGRAM_CACHE["runner"] = run
    return run


def kernel(**inputs):
    xyz = np.asarray(inputs["xyz"], np.float32)
    in_maps = host_prep(xyz)
    if "runner" not in _PROGRAM_CACHE:
        # first call: warm the NEFF via the stock SPMD path, then build
        # the cached runner for subsequent calls
        from concourse.bass_utils import run_bass_kernel_spmd
        nc = _get_program()
        results = run_bass_kernel_spmd(
            nc, in_maps, core_ids=list(range(NCORES))).results
        _get_runner()
    else:
        results = _get_runner()(in_maps)
    return host_post(results,
                     np.asarray(inputs["bn1_gamma"], np.float32),
                     np.asarray(inputs["bn1_beta"], np.float32),
                     np.asarray(inputs["bn2_gamma"], np.float32),
                     np.asarray(inputs["bn2_beta"], np.float32))


# revision 37
# speedup vs baseline: 2.9527x; 2.9527x over previous
"""PointTDA Trainium2 kernel: FPS + kNN + pooling on 8 NeuronCores.

Self-contained: builds a Bass/Tile program, shards batch 2-per-core,
runs via run_bass_kernel_spmd, finishes the tiny BN/cdist tail on host.

Device outputs per core (per sample s, coord c):
  nx    [6*S]            FPS centroid coords, col = 6*t + 2*c + s
  stats [2,3,2,128,S/8]  seg-reduced max/sum of gathered neighbor coords
  m8    [2,128,S/4]      top-32 "NS score" values per query (for sum of
                         selected neighbor distances on host)
where NS[q, n] = 2*(nx_q . x_n) - |x_n|^2  (monotone in -distance per row).
"""
import sys

sys.path.insert(0, "/opt/trn_rl_repo")

import numpy as np

# Problem constants (hardcoded per harness contract)
B, N, S, K = 16, 8192, 1024, 32
NCORES = 8
SPC = B // NCORES          # samples per core
EPS_BN = 1e-5
FACTOR = 1.0

_PROGRAM_CACHE = {}


def build_program(n=N, s_pts=S, k=K, spc=SPC, unroll=8, nschunk=512):
    """Build the Bacc program. Parameterized so a scaled-down version can be
    simulated in CoreSim. Returns the compiled Bacc object."""
    import concourse.bass as bass
    import concourse.bacc as bacc
    import concourse.bass_isa as bass_isa
    import concourse.mybir as mybir
    import concourse.tile as tile

    F32 = mybir.dt.float32
    U16 = mybir.dt.uint16
    ALU = mybir.AluOpType
    ACTF = mybir.ActivationFunctionType
    AXX = mybir.AxisListType.X

    P = 128                      # partitions
    FP = n // P                  # free elems per partition in FPS layout
    nrounds = k // 8
    nqt = s_pts // P             # query tiles per sample
    nch = n // nschunk           # NS chunks per row-tile
    ngrp = P // 16               # 16-partition gpsimd groups

    nc = bacc.Bacc("TRN2", target_bir_lowering=False, debug=False,
                   num_devices=NCORES)

    # ---- DRAM I/O ----
    # Single input blob per core to minimize axon transfer round-trips:
    # cols [0:3*spc*FP] = xyz2 (FPS layout), cols [3*spc*FP:] = iotb
    # (16384 - global point index, for the FPS argmax tie-break).
    blob_d = nc.dram_tensor("blob", [P, (3 * spc + 1) * FP], F32,
                            kind="ExternalInput")
    xyz2_d = blob_d[:, 0:3 * spc * FP]
    iotb_d = blob_d[:, 3 * spc * FP:(3 * spc + 1) * FP]
    # Single output blob: [nx | stats | m8sums]
    onx = 6 * s_pts
    ost = spc * 3 * 2 * (P // 16) * 16 * nqt
    om8 = spc * P * nqt
    outb_d = nc.dram_tensor("outb", [onx + ost + om8], F32,
                            kind="ExternalOutput")
    nx_d = outb_d[0:onx]
    idxd_d = nc.dram_tensor("idxd", [P, k * nqt], U16)   # internal scratch

    with tile.TileContext(nc) as tc:
        with tc.tile_pool(name="glob", bufs=1) as gp:
            # kNN moving tensors derived on device from the input blob.
            # Both samples in one tile at base partitions 0 / 32 (PE matmul
            # requires stationary/moving base partition in {0, 32, 64}).
            # Rows 32s+0..2 = coords (point-major), row 32s+3 = |x|^2.
            xyzt4 = gp.tile([36, n], F32)
            blobv = blob_d.rearrange("p (j f) -> p j f", f=FP)
            for s in range(spc):
                for c in range(3):
                    nc.sync.dma_start(xyzt4[32 * s + c:32 * s + c + 1, :],
                                      blobv[:, 2 * c + s, :])

            # ---------------- FPS ----------------
            with tc.tile_pool(name="fps", bufs=1) as fp:
                xyz2 = fp.tile([P, 3, spc, FP], F32)
                d2 = fp.tile([P, spc, FP], F32)
                s2 = fp.tile([P, 3, spc, FP], F32)
                t2 = fp.tile([P, spc, FP], F32)
                m2 = fp.tile([P, spc], F32)
                gm = fp.tile([P, spc], F32)
                sel = fp.tile([P, spc, FP], F32)
                m2i = fp.tile([P, spc], F32)
                gsel = fp.tile([P, spc], F32)
                iotb = fp.tile([P, FP], F32)
                msk = fp.tile([P, spc, FP], F32)
                prd = fp.tile([P, 3, spc, FP], F32)
                red = fp.tile([P, 3 * spc], F32)
                cent = fp.tile([P, 3 * spc], F32)
                nxacc = fp.tile([P, 6 * s_pts], F32)

                nc.sync.dma_start(xyz2[:], xyz2_d)
                nc.sync.dma_start(iotb[:], iotb_d)
                nc.vector.memset(d2[:], 1e10)
                # step-0 centroid = coords of point index 0 (partition 0, f 0)
                nc.gpsimd.partition_broadcast(
                    cent[:],
                    xyz2[0:1, :, :, 0:1].rearrange("o c s f -> o (c s f)"),
                    channels=P)

                def fps_body(iv):
                    base = iv * (3 * spc)
                    # record current centroid (dynamic OUT is HW-safe;
                    # dynamic bias/in operands are NOT)
                    nc.scalar.activation(
                        nxacc[:, bass.ds(base, 3 * spc)], cent[:],
                        ACTF.Copy, scale=1.0)
                    # squares (ACT): s2 = (cent - x)^2, bias from static cols
                    for j in range(3 * spc):
                        nc.scalar.activation(
                            s2[:].rearrange("p c s f -> p (c s) f")[:, j, :],
                            xyz2[:].rearrange("p c s f -> p (c s) f")[:, j, :],
                            ACTF.Square,
                            bias=cent[:, j:j + 1], scale=-1.0)
                    # dist = sx + sy + sz (DVE), min-update (POOL)
                    nc.vector.tensor_tensor(t2[:], s2[:, 0], s2[:, 1], ALU.add)
                    nc.vector.tensor_tensor(t2[:], t2[:], s2[:, 2], ALU.add)
                    nc.vector.tensor_tensor(d2[:], d2[:], t2[:], ALU.min)
                    # argmax: per-partition max, cross-partition allreduce
                    nc.vector.tensor_reduce(m2[:], d2[:], axis=AXX, op=ALU.max)
                    nc.gpsimd.partition_all_reduce(
                        gm[:], m2[:], channels=P,
                        reduce_op=bass_isa.ReduceOp.max)
                    # tie-break to lowest global index: sel = (D>=gmax)*(B-n)
                    for s in range(spc):
                        nc.vector.scalar_tensor_tensor(
                            sel[:, s], d2[:, s], gm[:, s:s + 1], iotb[:],
                            op0=ALU.is_ge, op1=ALU.mult)
                    nc.vector.tensor_reduce(m2i[:], sel[:], axis=AXX,
                                            op=ALU.max)
                    nc.gpsimd.partition_all_reduce(
                        gsel[:], m2i[:], channels=P,
                        reduce_op=bass_isa.ReduceOp.max)
                    # extract winning point's coords via mask + allreduce-add
                    nc.vector.tensor_tensor(
                        msk[:], sel[:],
                        gsel[:].unsqueeze(-1).broadcast_to([P, spc, FP]),
                        ALU.is_ge)
                    nc.vector.tensor_tensor(
                        prd[:], xyz2[:],
                        msk[:].unsqueeze(1).broadcast_to([P, 3, spc, FP]),
                        ALU.mult)
                    nc.vector.tensor_reduce(
                        red[:].rearrange("p (c s) -> p c s", c=3),
                        prd[:], axis=AXX, op=ALU.add)
                    nc.gpsimd.partition_all_reduce(
                        cent[:], red[:], channels=P,
                        reduce_op=bass_isa.ReduceOp.add)

                tc.For_i_unrolled(0, s_pts, 1, fps_body, max_unroll=unroll)
                # dump centroids (row 0; all rows identical)
                nc.sync.dma_start(nx_d, nxacc[0:1, 0:6 * s_pts])

            # ---------------- kNN + pooling ----------------
            with tc.tile_pool(name="knn", bufs=1) as kp, \
                 tc.tile_pool(name="knn2", bufs=2) as kp2, \
                 tc.tile_pool(name="psum", bufs=4, space="PSUM") as pp:
                ns = kp.tile([P, n], F32)
                xbc = [kp.tile([P, n], F32, tag=f"xbc{c}", name=f"xbc{c}")
                       for c in range(3)]
                ones1 = kp.tile([1, P], F32)
                nc.vector.memset(ones1[:], 1.0)

                nxt4 = kp.tile([36, s_pts], F32)
                sq3 = kp.tile([35, n], F32)
                ones3 = kp.tile([35, 1], F32)
                nc.vector.memset(ones3[0:3, :], 1.0)
                if spc > 1:
                    nc.vector.memset(ones3[32:35, :], 1.0)
                for s in range(spc):
                    # |x|^2 row of the moving tensor: squares + 3-partition
                    # ones-matmul, staged through ns[0:1]
                    nc.vector.tensor_tensor(sq3[32 * s:32 * s + 3, :],
                                            xyzt4[32 * s:32 * s + 3, :],
                                            xyzt4[32 * s:32 * s + 3, :],
                                            ALU.mult)
                    for ch in range(nch):
                        pn = pp.tile([1, nschunk], F32, tag="pn")
                        sl = slice(ch * nschunk, (ch + 1) * nschunk)
                        nc.tensor.matmul(pn[:], ones3[32 * s:32 * s + 3, :],
                                         sq3[32 * s:32 * s + 3, sl],
                                         start=True, stop=True)
                        nc.scalar.activation(ns[0:1, sl], pn[:],
                                             ACTF.Copy, scale=1.0)
                    nc.sync.dma_start(xyzt4[32 * s + 3:32 * s + 4, :],
                                      ns[0:1, :])

                    nxs = nxt4[32 * s:32 * s + 4, :]
                    nxv = nx_d.rearrange("(t c s) -> s c t", t=s_pts, c=3,
                                            s=spc)
                    nc.sync.dma_start(nxs[0:3, :], nxv[s])
                    # row 3 of the stationary = -0.5 (scales |x|^2 row)
                    nc.vector.memset(ns[0:1, 0:s_pts], -0.5)
                    nc.sync.dma_start(nxs[3:4, :], ns[0:1, 0:s_pts])

                    # replicated coord arrays for gathers (stage coord rows
                    # through ns[0:1], which is free until the NS chunks)
                    for c in range(3):
                        nc.sync.dma_start(ns[0:1, :],
                                          xyzt4[32 * s + c:32 * s + c + 1, :])
                        for ch in range(nch):
                            pb = pp.tile([P, nschunk], F32, tag="ps")
                            sl = slice(ch * nschunk, (ch + 1) * nschunk)
                            nc.tensor.matmul(pb[:], ones1[:], ns[0:1, sl],
                                             start=True, stop=True)
                            nc.scalar.activation(xbc[c][:, sl], pb[:],
                                                 ACTF.Copy, scale=1.0)

                    idx = kp.tile([P, k * nqt], U16, tag="idx")
                    iw = kp.tile([P, k * nqt], U16, tag="iw")
                    m8b = kp.tile([P, k * nqt], F32, tag="m8b")
                    m8s = kp.tile([P, nqt], F32, tag="m8s")
                    rstat = [[kp.tile([P, 16 * nqt], F32, tag=f"rs{c}_{st}",
                                      name=f"rs{c}_{st}")
                              for st in range(2)] for c in range(3)]

                    for t in range(nqt):
                        qs = slice(t * P, (t + 1) * P)
                        for ch in range(nch):
                            pb = pp.tile([P, nschunk], F32, tag="ps")
                            sl = slice(ch * nschunk, (ch + 1) * nschunk)
                            nc.tensor.matmul(pb[:], nxs[:, qs],
                                             xyzt4[32 * s:32 * s + 4, sl],
                                             start=True, stop=True)
                            nc.vector.tensor_scalar(ns[:, sl], pb[:], 2.0,
                                                    None, ALU.mult)
                        # top-k rounds (match_replace in place)
                        for r in range(nrounds):
                            mv = m8b[:, k * t + 8 * r: k * t + 8 * r + 8]
                            nc.vector.max(mv, ns[:])
                            nc.vector.max_index(
                                idx[:, k * t + 8 * r: k * t + 8 * r + 8],
                                mv, ns[:])
                            if r < nrounds - 1:
                                nc.vector.match_replace(ns[:], mv, ns[:],
                                                        -3e38)
                        # per-query sum of the top-k scores (for host std)
                        nc.vector.tensor_reduce(
                            m8s[:, t:t + 1], m8b[:, k * t:k * (t + 1)],
                            axis=AXX, op=ALU.add)

                    # wrap indices into gpsimd group-shared layout via DRAM
                    nc.sync.dma_start(idxd_d[:], idx[:])
                    njh = k // 16
                    for t in range(nqt):
                        rsrc = idxd_d[:].rearrange(
                            "(g l) (t jh jl) -> g jl (t jh) l",
                            g=ngrp, l=16, t=nqt, jh=njh, jl=16)
                        rdst = iw[:].rearrange(
                            "p (t l jh) -> p t jh l", t=nqt, l=16, jh=njh)
                        for jh in range(njh):
                            for g in range(ngrp):
                                nc.sync.dma_start(
                                    rdst[16 * g:16 * (g + 1), t, jh, :],
                                    rsrc[g, :, t * njh + jh, :])

                    for t in range(nqt):
                        isl = iw[:, k * t: k * (t + 1)]
                        for c in range(3):
                            g = kp2.tile([P, 16 * k], F32, tag="g")
                            nc.gpsimd.indirect_copy(
                                g[:], xbc[c][:], isl,
                                i_know_ap_gather_is_preferred=True)
                            gv = g[:].rearrange("p (j kk) -> p j kk", j=16)
                            nc.vector.tensor_reduce(
                                rstat[c][0][:, 16 * t:16 * (t + 1)], gv,
                                axis=AXX, op=ALU.max)
                            nc.vector.tensor_reduce(
                                rstat[c][1][:, 16 * t:16 * (t + 1)], gv,
                                axis=AXX, op=ALU.add)

                    seglen = (P // 16) * 16 * nqt
                    for c in range(3):
                        for st in range(2):
                            # one row per 16-partition group (dedup the 16x
                            # replication before download)
                            srca = rstat[c][st][:].rearrange(
                                "(g l) f -> g l f", l=16)[:, 0:1, :]
                            off = onx + (((s * 3 + c) * 2 + st) * seglen)
                            nc.sync.dma_start(
                                outb_d[off:off + seglen].rearrange(
                                    "(g f) -> g f", g=P // 16), srca)
                    moff = onx + ost + s * P * nqt
                    nc.sync.dma_start(
                        outb_d[moff:moff + P * nqt].rearrange(
                            "(p t) -> p t", p=P), m8s[:])

    nc.compile()
    return nc


def _get_program():
    if "full" not in _PROGRAM_CACHE:
        _PROGRAM_CACHE["full"] = build_program()
    return _PROGRAM_CACHE["full"]


def host_prep(xyz):
    """Per-core input blobs. xyz: [B, N, 3] float32."""
    fp = N // 128
    iotb = (16384.0 - (np.arange(128)[:, None] * fp
                       + np.arange(fp)[None, :])).astype(np.float32)
    in_maps = []
    for core in range(NCORES):
        xs = xyz[SPC * core: SPC * (core + 1)]          # [spc, N, 3]
        a = xs.reshape(SPC, 128, fp, 3)                 # s p f c
        blob = np.empty((128, (3 * SPC + 1) * fp), np.float32)
        blob[:, 0:3 * SPC * fp] = np.transpose(a, (1, 3, 0, 2)).reshape(
            128, 3 * SPC * fp)
        blob[:, 3 * SPC * fp:] = iotb
        in_maps.append({"blob": blob})
    return in_maps


def host_post(results, bn1_gamma, bn1_beta, bn2_gamma, bn2_beta,
              n=N, s_pts=S, k=K):
    """Combine per-core device outputs into the final [B, 36] features."""
    nqt = s_pts // 128
    onx = 6 * s_pts
    ost = SPC * 3 * 2 * 8 * 16 * nqt
    nxs, maxg, sumg, m8v = [], [], [], []
    for r in results:
        outb = r["outb"]
        nx = outb[0:onx].reshape(s_pts, 3, SPC)          # t c s
        stats = outb[onx:onx + ost].reshape(SPC, 3, 2, 8, 16 * nqt)
        m8 = outb[onx + ost:].reshape(SPC, 128, nqt)
        for s in range(SPC):
            nxs.append(nx[:, :, s])                      # [S, 3]
            # stats group row g, col 16t+j -> query 128t+16g+j
            st = stats[s].reshape(3, 2, 8, nqt, 16)
            st = np.transpose(st, (0, 1, 3, 2, 4)).reshape(3, 2, s_pts)
            maxg.append(st[:, 0].T)                      # [S, 3]
            sumg.append(st[:, 1].T)
            # m8 sums: row p, col t -> query 128t+p
            m8v.append(m8[s].T.reshape(s_pts))

    nx = np.stack(nxs)            # [B, S, 3]
    mxg = np.stack(maxg)          # [B, S, 3]
    smg = np.stack(sumg)          # [B, S, 3]
    m8v = np.stack(m8v)           # [B, S] sum of top-k scores

    # global std of diff (ddof=1) from sums
    sum_d = (smg - k * nx).sum(dtype=np.float64)
    nx2 = (nx.astype(np.float64) ** 2).sum(-1)           # |a|^2 [B,S]
    sum_d2 = (k * nx2 - m8v.astype(np.float64)).sum()
    M = B * s_pts * k * 3
    var = (sum_d2 - sum_d * sum_d / M) / (M - 1)
    std = np.sqrt(max(var, 0.0))

    maxdiff = mxg - nx
    meandiff = smg / k - nx
    p3 = (maxdiff + meandiff) / (std + 1e-5)
    lc = np.concatenate([p3, 2.0 * nx], axis=2)          # [B, S, 6]
    lc = np.transpose(lc, (0, 2, 1)).astype(np.float32)  # [B, 6, S]

    mean = lc.mean(axis=(0, 2), keepdims=True)
    varr = lc.var(axis=(0, 2), keepdims=True)
    lc = (lc - mean) / np.sqrt(varr + EPS_BN) \
        * bn1_gamma[None, :, None] + bn1_beta[None, :, None]
    lc = np.maximum(lc, 0.0)

    # safe cdist over channel rows -> [B, 6, 6]
    x64 = lc.astype(np.float64)
    g = np.einsum("bcs,bds->bcd", x64, x64)
    sq = np.maximum(g.diagonal(axis1=1, axis2=2)[:, :, None]
                    + g.diagonal(axis1=1, axis2=2)[:, None, :] - 2 * g, 0.0)
    # recompute exactly like reference for better fidelity
    d = x64[:, :, None, :] - x64[:, None, :, :]
    sq = (d * d).sum(-1)
    tfcw = np.where(sq > 0, np.sqrt(np.where(sq > 0, sq, 1.0)), 0.0)
    i = np.arange(5)
    tfcw[:, i, i + 1] *= FACTOR

    feat = tfcw.reshape(B, -1)
    mean2 = feat.mean(axis=0, keepdims=True)
    var2 = feat.var(axis=0, keepdims=True)
    feat = (feat - mean2) / np.sqrt(var2 + EPS_BN) \
        * bn2_gamma[None, :] + bn2_beta[None, :]
    feat = np.maximum(feat, 0.0)
    return feat.astype(np.float32)


def _get_runner():
    """Cached jitted shard_map runner over the 8 cores (mirrors
    run_bass_kernel_spmd's axon path, but without per-call retracing)."""
    if "runner" in _PROGRAM_CACHE:
        return _PROGRAM_CACHE["runner"]

    import jax
    import concourse.mybir as mybir
    from concourse import bass2jax
    from jax.sharding import Mesh, PartitionSpec
    from jax.experimental.shard_map import shard_map

    nc = _get_program()
    bass2jax.install_neuronx_cc_hook()
    partition_name = (nc.partition_id_tensor.name
                      if nc.partition_id_tensor else None)
    in_names, out_names, out_avals, zero_shapes = [], [], [], []
    for alloc in nc.m.functions[0].allocations:
        if not isinstance(alloc, mybir.MemoryLocationSet):
            continue
        name = alloc.memorylocations[0].name
        if alloc.kind == "ExternalInput":
            if name != partition_name:
                in_names.append(name)
        elif alloc.kind == "ExternalOutput":
            out_names.append(name)
            shape = tuple(alloc.tensor_shape)
            dtype = mybir.dt.np(alloc.dtype)
            out_avals.append(jax.core.ShapedArray(shape, dtype))
            zero_shapes.append((shape, dtype))
    n_params = len(in_names)
    n_outs = len(out_avals)
    in_names_full = (in_names + out_names
                     + ([partition_name] if partition_name else []))
    donate = tuple(range(n_params, n_params + n_outs))

    def _body(*args):
        operands = list(args)
        if partition_name is not None:
            operands.append(bass2jax.partition_id_tensor())
        outs = bass2jax._bass_exec_p.bind(
            *operands, out_avals=tuple(out_avals),
            in_names=tuple(in_names_full), out_names=tuple(out_names),
            lowering_input_output_aliases=(), sim_require_finite=True,
            sim_require_nnan=True, nc=nc)
        return tuple(outs)

    devices = jax.devices()[:NCORES]
    mesh = Mesh(np.asarray(devices), ("core",))
    sharded = jax.jit(
        shard_map(_body, mesh=mesh,
                  in_specs=(PartitionSpec("core"),) * (n_params + n_outs),
                  out_specs=(PartitionSpec("core"),) * n_outs,
                  check_rep=False),
        donate_argnums=donate, keep_unused=True)

    def run(in_maps):
        concat_in = [np.concatenate([np.asarray(in_maps[c][nm])
                                     for c in range(NCORES)], axis=0)
                     for nm in in_names]
        zeros = [np.zeros((NCORES * sh[0], *sh[1:]), dt)
                 for sh, dt in zero_shapes]
        out = sharded(*concat_in, *zeros)
        return [{name: np.asarray(out[i]).reshape(NCORES,
                                                  *out_avals[i].shape)[c]
                 for i, name in enumerate(out_names)}
                for c in range(NCORES)]

    _PROGRAM_CACHE["runner"] = run
    return run


def kernel(**inputs):
    xyz = np.asarray(inputs["xyz"], np.float32)
    in_maps = host_prep(xyz)
    if "runner" not in _PROGRAM_CACHE:
        # first call: warm the NEFF via the stock SPMD path, then build
        # the cached runner for subsequent calls
        from concourse.bass_utils import run_bass_kernel_spmd
        nc = _get_program()
        results = run_bass_kernel_spmd(
            nc, in_maps, core_ids=list(range(NCORES))).results
        _get_runner()
    else:
        results = _get_runner()(in_maps)
    return host_post(results,
                     np.asarray(inputs["bn1_gamma"], np.float32),
                     np.asarray(inputs["bn1_beta"], np.float32),
                     np.asarray(inputs["bn2_gamma"], np.float32),
                     np.asarray(inputs["bn2_beta"], np.float32))
